# revision 2
# baseline (speedup 1.0000x reference)
"""BaselineGNN (SAGEConv-mean x3 + BN + relu, graph mean-pool, MLP head) on 8 Trainium2 cores.

V2 strategy (vs baseline):
  - Nodes/edges sharded by graph across 8 cores (each core owns 512 graphs' dst nodes).
  - The replicated node-feature table is split into 4 interleaved chunks
    (chunk k = quarter k of every core's shard) so the inter-layer AllGather is
    4 pipelined chunk-collectives, each unblocking that chunk's gathers.
  - Edges sorted by (src chunk, dst); per-(chunk,window) bucket sizes shared
    across cores (max) so the 8 cores run one SPMD schedule (~6% pad only).
  - Tiles of 128 edges span at most 2 dst windows of 128; each tile is 1-2
    matmuls into per-window PSUM accumulators (rhs = 256-wide one-hot built by
    DVE is_equal against iota).
  - No per-edge scaling: mean-divide folded into one per-node multiply of the
    f32 agg buffer (inv-degree broadcast tile).
  - BN batch stats via ScalarE accumulators + [128,2] AllReduce; BN apply+relu
    fused; per-quarter transpose + AllGather launch for the next layer.
"""
import os
import numpy as np
import ml_dtypes

from concourse import bass, bacc, mybir
from concourse.bass_utils import run_bass_kernel_spmd
from concourse.masks import make_identity
import concourse.tile as tile

BF16 = mybir.dt.bfloat16
F32 = mybir.dt.float32
I16 = mybir.dt.int16
I32 = mybir.dt.int32

C = 8            # cores
D = 128          # feature dim
HD = 64          # head hidden dim
L = 3            # layers
G = 4096         # graphs
W = 128          # dst window width
TILE = 128       # edges per matmul tile
CALL = 2048      # max indices per dma_gather call
NK = 4           # table chunks (= src quarters)
BLK = 512        # node block for update matmuls
BN_EPS = 1e-5

LAST_RESULT = None


def _ceil(a, b):
    return -(-a // b) * b


class Plan:
    pass


def _preprocess(x, esrc, edst, bids):
    p = Plan()
    N = x.shape[0]
    GPC = G // C
    p.N, p.GPC = N, GPC

    node_start = np.searchsorted(bids, np.arange(0, G + 1, GPC)).astype(np.int64)
    n_c = np.diff(node_start)
    PN = _ceil(int(n_c.max()), 512)
    PNQ = PN // NK
    CS = C * PNQ
    assert CS <= 32767
    p.PN, p.PNQ, p.CS = PN, PNQ, CS
    p.NB = PN // 128
    p.NW = PN // W

    own = np.repeat(np.arange(C), n_c)
    local = np.arange(N) - node_start[own]

    deg = np.bincount(edst, minlength=N).astype(np.float32)
    invdeg = (1.0 / np.maximum(deg, 1.0)).astype(np.float32)

    e_own = own[edst]
    e_dl = local[edst]
    e_k = local[esrc] // PNQ
    e_ci = (own[esrc] * PNQ + (local[esrc] % PNQ)).astype(np.int64)

    NW = p.NW
    # shared bucket sizes with smoothing: a core whose (k, w) count exceeds
    # the shared budget spills its tail edges into bucket w-1's slack (they
    # land in tiles with w1 = w-1 and use the second-half one-hot)
    key = ((e_own * NK + e_k) * NW + e_dl // W).astype(np.int64)
    counts = np.bincount(key, minlength=C * NK * NW).reshape(C, NK, NW)
    B = np.zeros((NK, NW), np.int64)
    spill = np.zeros((C, NK, NW), np.int64)   # edges of (c,k,w) placed in w-1
    ms = np.zeros((NK, NW + 1), np.int64)     # max spill zone size at top of w-1
    for k in range(NK):
        for w in range(NW):
            cnt = counts[:, k, w]
            if w == 0:
                B[k, 0] = cnt.max()
                continue
            # slack in bucket w-1 per core: budget minus its kept edges there
            kept_prev = counts[:, k, w - 1] - spill[:, k, w - 1]
            slack = B[k, w - 1] - kept_prev
            spillable = np.minimum(np.maximum(slack, 0), cnt)
            Bw = int((cnt - spillable).max())
            sp = np.maximum(cnt - Bw, 0)
            assert (sp <= spillable).all()
            B[k, w] = Bw
            spill[:, k, w] = sp
            ms[k, w] = int(sp.max())
    p.spill = spill

    # shared stream layout: per phase k, bucket offsets; phase padded to TILE
    ph_size = np.zeros(NK + 1, np.int64)
    boff = np.zeros((NK, NW + 1), np.int64)
    for k in range(NK):
        boff[k, 1:] = np.cumsum(B[k])
        ph_size[k + 1] = ph_size[k] + _ceil(int(boff[k, NW]), TILE)
    EP = int(ph_size[NK])
    p.EP = EP
    p.ph_size = ph_size

    # shared tile schedule: per tile, phase + w1 + crossing
    tiles = []   # (k, w1, crossing)
    mm_sched = []  # per tile: list of (window, start, stop) halves
    win_mm = {}  # (k, w) -> list of (tile_idx, half)
    for k in range(NK):
        nt = (ph_size[k + 1] - ph_size[k]) // TILE
        for t in range(nt):
            lo = t * TILE          # position within phase
            hi = lo + TILE
            w1 = int(np.searchsorted(boff[k], lo, side="right") - 1)
            w1 = min(w1, NW - 1)
            # crossing if the tile extends beyond bucket w1 OR reaches the
            # spill zone at the top of w1 (holding w1+1-dst edges)
            cross = (w1 + 1 < NW and
                     hi > boff[k, w1 + 1] - ms[k, w1 + 1])
            if cross and w1 + 2 < NW:
                assert hi <= boff[k, w1 + 2] - ms[k, w1 + 2], (
                    "tile spans >2 windows (spill)")
            ti = len(tiles)
            tiles.append((k, w1, cross))
            win_mm.setdefault((k, w1), []).append((ti, 0))
            if cross:
                win_mm.setdefault((k, w1 + 1), []).append((ti, 1))
    p.tiles = tiles
    # mark start/stop: for each (k,w) group, first and last mm
    p.mm_flags = {}
    p.flush_after = {}   # tile idx -> list of (k, w) groups that close there
    for (k, w), mms in win_mm.items():
        for j, (ti, half) in enumerate(mms):
            p.mm_flags[(ti, half)] = (j == 0, j == len(mms) - 1)
        last_ti = mms[-1][0]
        p.flush_after.setdefault(last_ti, []).append((k, w))
    p.win_mm = win_mm

    # gather calls (shared): per phase, groups of <= CALL idxs
    calls = []   # (k, p0 global slot offset, n)
    for k in range(NK):
        pos = int(ph_size[k])
        end = int(ph_size[k + 1])
        while pos < end:
            n = min(CALL, end - pos)
            calls.append((k, pos, n))
            pos += n
    p.calls = calls

    # crossing tiles get their second-half one-hots from a dense ed2 array
    cross_tiles = [ti for ti, (k, w1, cr) in enumerate(tiles) if cr]
    cross_col = {ti: j for j, ti in enumerate(cross_tiles)}
    p.cross_tiles, p.cross_col = cross_tiles, cross_col
    p.NC2 = max(len(cross_tiles), 1)

    # per-core data arrays
    p.eidx, p.ed, p.ed2 = [], [], []
    p.xt0, p.invb = [], []
    p.wpool, p.bloc, p.mask_tail = [], [], []
    cnt = np.bincount(bids, minlength=G).astype(np.float32)
    inv_cnt = (1.0 / np.maximum(cnt, 1.0)).astype(np.float32)
    MT = min(PN, 1024)
    p.MT = MT

    # tile w1 per global slot (for ed computation)
    slot_w1 = np.empty(EP, np.int64)
    for ti, (k, w1, cross) in enumerate(tiles):
        slot_w1[ti * TILE:(ti + 1) * TILE] = w1

    for c in range(C):
        sel = np.where(e_own == c)[0]
        order = sel[np.lexsort((e_ci[sel], e_dl[sel] // W, e_k[sel]))]
        ks = e_k[order]
        ws = e_dl[order] // W
        # position of each edge: phase base + bucket offset + rank in bucket
        kw = ks * NW + ws
        bstart = np.searchsorted(kw, np.arange(NK * NW))
        rank = np.arange(len(order)) - bstart[kw]
        cnt_e = counts[c].reshape(-1)[kw]
        sp_e = spill[c].reshape(-1)[kw]
        keep_e = cnt_e - sp_e
        # kept edges at bucket start; spilled tail at top of bucket w-1
        pos = ph_size[ks] + boff[ks, ws] + np.where(
            rank < keep_e, rank, rank - cnt_e)
        assert (pos >= 0).all()
        idx_arr = np.zeros(EP, np.int16)
        ed_arr = np.full(EP, -1.0, np.float32)
        idx_arr[pos] = e_ci[order].astype(np.int16)
        ed_arr[pos] = (e_dl[order] - slot_w1[pos] * W).astype(np.float32)
        assert ed_arr.max() < 2 * W
        eidx16 = np.empty((16, EP // 16), np.int16)
        for (k, p0, n) in calls:
            eidx16[:, p0 // 16:(p0 + n) // 16] = \
                idx_arr[p0:p0 + n].reshape(n // 16, 16).T
        p.eidx.append(np.tile(eidx16, (8, 1)))
        edT = ed_arr.reshape(EP // 128, 128).T.astype(ml_dtypes.bfloat16)
        p.ed.append(edT)
        ed2 = np.full((128, p.NC2), -1.0, ml_dtypes.bfloat16)
        if cross_tiles:
            ed2[:, :len(cross_tiles)] = edT[:, cross_tiles]
        p.ed2.append(ed2)

        nc_ = int(n_c[c])
        xt = np.zeros((D, PN), ml_dtypes.bfloat16)
        xt[:, :nc_] = x[node_start[c]:node_start[c + 1]].T.astype(ml_dtypes.bfloat16)
        p.xt0.append(xt)
        iv = np.zeros(PN, np.float32)
        iv[:nc_] = invdeg[node_start[c]:node_start[c + 1]]
        p.invb.append(np.tile(iv[None, :], (128, 1)).astype(ml_dtypes.bfloat16))
        wp = np.zeros(PN, np.float32)
        bl = np.full(PN, -1.0, np.float32)
        gids = bids[node_start[c]:node_start[c + 1]]
        wp[:nc_] = inv_cnt[gids]
        bl[:nc_] = (gids - c * GPC).astype(np.float32)
        p.wpool.append(wp.reshape(PN // 128, 128).T.copy())
        p.bloc.append(bl.reshape(PN // 128, 128).T.copy())
        mt = np.zeros(MT, ml_dtypes.bfloat16)
        valid_in_tail = nc_ - (PN - MT)
        if valid_in_tail > 0:
            mt[:valid_in_tail] = 1.0
        p.mask_tail.append(np.tile(mt[None, :], (128, 1)))

    # layer-0 chunk tables (shared across cores)
    p.t0 = []
    for k in range(NK):
        t = np.zeros((CS, D), ml_dtypes.bfloat16)
        for c in range(C):
            lo = k * PNQ
            hi = min((k + 1) * PNQ, int(n_c[c]))
            if hi > lo:
                rows = x[node_start[c] + lo:node_start[c] + hi]
                t[c * PNQ:c * PNQ + (hi - lo)] = rows.astype(ml_dtypes.bfloat16)
        p.t0.append(t)
    return p


def _build(p):
    PN, PNQ, CS, EP, NW, NB = p.PN, p.PNQ, p.CS, p.EP, p.NW, p.NB
    GPC = p.GPC
    NBLK = PN // BLK
    NQB = PNQ // 128      # 128-blocks per quarter
    nc = bacc.Bacc('TRN2', target_bir_lowering=False, debug=False,
                   num_devices=C, num_swdge_queues=4,
                   dynamic_dma_scratch_size=32768)

    # ---- parameters ----
    t0 = [nc.declare_dram_parameter(f"t0_{k}", [CS, D], BF16, isOutput=False)
          for k in range(NK)]
    xt0 = nc.declare_dram_parameter("xt0", [D, PN], BF16, isOutput=False)
    eidx = nc.declare_dram_parameter("eidx", [128, EP // 16], I16, isOutput=False)
    ed = nc.declare_dram_parameter("ed", [128, EP // 128], BF16, isOutput=False)
    ed2 = nc.declare_dram_parameter("ed2", [128, p.NC2], BF16, isOutput=False)
    invb = nc.declare_dram_parameter("invb", [128, PN], BF16, isOutput=False)
    wl_p = nc.declare_dram_parameter("wl", [L, D, D], BF16, isOutput=False)
    wr_p = nc.declare_dram_parameter("wr", [L, D, D], BF16, isOutput=False)
    gb_p = nc.declare_dram_parameter("gb", [D, L, 2], F32, isOutput=False)
    wpool_p = nc.declare_dram_parameter("wpool", [128, NB], F32, isOutput=False)
    bloc_p = nc.declare_dram_parameter("bloc", [128, NB], F32, isOutput=False)
    mtail_p = nc.declare_dram_parameter("mtail", [128, p.MT], BF16, isOutput=False)
    w1_p = nc.declare_dram_parameter("w1", [D, HD], BF16, isOutput=False)
    b1_p = nc.declare_dram_parameter("b1", [HD, 1], F32, isOutput=False)
    w2_p = nc.declare_dram_parameter("w2", [HD, 1], BF16, isOutput=False)
    b2_p = nc.declare_dram_parameter("b2", [1, 1], F32, isOutput=False)
    out_p = nc.declare_dram_parameter("out", [GPC], F32, isOutput=True)

    # ---- internal DRAM ----
    tabs = [t0]
    for l in range(1, L):
        tabs.append([nc.dram_tensor(f"t{l}_{k}", [CS, D], BF16,
                                    addr_space="Shared") for k in range(NK)])
    shardq = [[nc.dram_tensor(f"shard{l}_{k}", [PNQ, D], BF16)
               for k in range(NK)] for l in range(L - 1)]
    bnin = [nc.dram_tensor(f"bnin{l}", [D, 2], F32) for l in range(L)]
    bnout = [nc.dram_tensor(f"bnout{l}", [D, 2], F32, addr_space="Shared")
             for l in range(L)]
    rg = [list(range(C))]

    from contextlib import ExitStack
    with tile.TileContext(nc) as tc, ExitStack() as es:
        const = es.enter_context(tc.tile_pool(name="const", bufs=1))
        big = es.enter_context(tc.tile_pool(name="big", bufs=1))
        gp = es.enter_context(tc.tile_pool(name="gp", bufs=5))
        sp = es.enter_context(tc.tile_pool(name="sp", bufs=3))
        gsel = es.enter_context(tc.tile_pool(name="gsel", bufs=2))
        headp = es.enter_context(tc.tile_pool(name="headp", bufs=1))
        smallp = es.enter_context(tc.tile_pool(name="small", bufs=4))
        stage = es.enter_context(tc.tile_pool(name="stage", bufs=1))
        sqp = es.enter_context(tc.tile_pool(name="sqp", bufs=1))
        eip = es.enter_context(tc.tile_pool(name="eip", bufs=2))
        aggps = es.enter_context(tc.tile_pool(name="aggps", bufs=3, space="PSUM"))
        zps = es.enter_context(tc.tile_pool(name="zps", bufs=2, space="PSUM"))
        tps = es.enter_context(tc.tile_pool(name="tps", bufs=3, space="PSUM"))

        # ---- persistent constants ----
        iota_i = const.tile([128, 2 * W], I32)
        nc.gpsimd.iota(iota_i[:], pattern=[[1, 2 * W]], base=0, channel_multiplier=0)
        iota256 = const.tile([128, 2 * W], BF16)
        nc.vector.tensor_copy(out=iota256[:], in_=iota_i[:])
        iotaG_i = const.tile([128, GPC], I32)
        nc.gpsimd.iota(iotaG_i[:], pattern=[[1, GPC]], base=0, channel_multiplier=0)
        iotaG = const.tile([128, GPC], F32)
        nc.vector.tensor_copy(out=iotaG[:], in_=iotaG_i[:])
        ident = const.tile([128, 128], BF16)
        make_identity(nc, ident[:])

        wl_s = const.tile([128, L * D], BF16)
        wr_s = const.tile([128, L * D], BF16)
        for l in range(L):
            nc.sync.dma_start(out=wl_s[:, l * D:(l + 1) * D], in_=wl_p[l])
            nc.sync.dma_start(out=wr_s[:, l * D:(l + 1) * D], in_=wr_p[l])
        gb_s = const.tile([128, L, 2], F32)
        nc.sync.dma_start(out=gb_s[:], in_=gb_p[:])
        w1_s = const.tile([D, HD], BF16)
        nc.sync.dma_start(out=w1_s[:], in_=w1_p[:])
        b1_s = const.tile([HD, 1], F32)
        nc.sync.dma_start(out=b1_s[:], in_=b1_p[:])
        w2_s = const.tile([HD, 1], BF16)
        nc.sync.dma_start(out=w2_s[:], in_=w2_p[:])
        b2_s = const.tile([1, 1], F32)
        nc.sync.dma_start(out=b2_s[:], in_=b2_p[:])
        wpool_s = const.tile([128, NB], F32)
        nc.sync.dma_start(out=wpool_s[:], in_=wpool_p[:])
        bloc_s = const.tile([128, NB], F32)
        nc.sync.dma_start(out=bloc_s[:], in_=bloc_p[:])
        mtail_s = const.tile([128, p.MT], BF16)
        nc.sync.dma_start(out=mtail_s[:], in_=mtail_p[:])
        eps_s = const.tile([128, 1], F32)
        nc.vector.memset(eps_s[:], BN_EPS)

        # eidx is staged per (layer, phase) into a rotating buffer
        ph16 = []  # (lo16, n16) per phase
        pos16 = 0
        for k in range(NK):
            n16 = (p.ph_size[k + 1] - p.ph_size[k]) // 16
            ph16.append((int(p.ph_size[k]) // 16, int(n16)))
        PH16MAX = max(n16 for _, n16 in ph16)
        ed_s = big.tile([128, EP // 128], BF16, tag="ed")
        nc.sync.dma_start(out=ed_s[:], in_=ed[:])
        ed2_s = big.tile([128, p.NC2], BF16, tag="ed2")
        nc.sync.dma_start(out=ed2_s[:], in_=ed2[:])
        inv_s = big.tile([128, PN], BF16, tag="inv")
        nc.sync.dma_start(out=inv_s[:], in_=invb[:])

        xt = [big.tile([D, PN], BF16, tag="xt0", name="xt_a"),
              big.tile([D, PN], BF16, tag="xt1", name="xt_b")]
        nc.sync.dma_start(out=xt[0][:], in_=xt0[:])
        # one agg tile per quarter so the update phase can start on quarter 0
        # while later quarters are still aggregating
        aggq = [big.tile([D, PNQ], BF16, tag=f"agg{q}", name=f"agg_q{q}")
                for q in range(NK)]
        sq_scr = sqp.tile([128, BLK], F32, tag="sqscr")

        scope = nc.named_scope
        for l in range(L):
            xt_cur = xt[l % 2]
            xt_nxt = xt[(l + 1) % 2]

            NWQ = PNQ // W
            es_l = ExitStack(); es_l.enter_context(scope(f"agg{l}"))
            for q in range(NK):
                nc.vector.memset(aggq[q][:], 0.0)
            open_ps = {}
            cur_phase = -1
            eidx_p = None
            for ci, (k, p0, n) in enumerate(p.calls):
                if k != cur_phase:
                    cur_phase = k
                    lo16, n16 = ph16[k]
                    eidx_p = eip.tile([128, PH16MAX], I16, tag="eip")
                    nc.sync.dma_start(out=eidx_p[:, 0:n16],
                                      in_=eidx[:, lo16:lo16 + n16])
                    ph_lo16 = lo16
                T = n // 128
                t0i = p0 // 128      # first tile index of this call
                g = gp.tile([128, T, D], BF16, tag="g")
                nc.gpsimd.dma_gather(
                    out_ap=g[:],
                    in_ap=tabs[l][k].ap(),
                    idxs_ap=eidx_p[:, p0 // 16 - ph_lo16:(p0 + n) // 16 - ph_lo16],
                    num_idxs=n, num_idxs_reg=n, elem_size=D,
                    single_packet=(n <= 1024),
                    queue_num=ci % 4,
                )
                S = sp.tile([128, T, W], BF16, tag="S")
                nc.vector.tensor_tensor(
                    out=S[:],
                    in0=ed_s[:, t0i:t0i + T].unsqueeze(-1).to_broadcast([128, T, W]),
                    in1=iota256[:, 0:W].unsqueeze(1).to_broadcast([128, T, W]),
                    op=mybir.AluOpType.is_equal)
                # second halves for this call's crossing tiles, one build
                call_cross = [ti for ti in range(t0i, t0i + T) if p.tiles[ti][2]]
                S2 = None
                if call_cross:
                    cx0 = p.cross_col[call_cross[0]]
                    Tc = len(call_cross)
                    S2 = sp.tile([128, Tc, W], BF16, tag="S2")
                    nc.vector.tensor_tensor(
                        out=S2[:],
                        in0=ed2_s[:, cx0:cx0 + Tc].unsqueeze(-1).to_broadcast([128, Tc, W]),
                        in1=iota256[:, W:2 * W].unsqueeze(1).to_broadcast([128, Tc, W]),
                        op=mybir.AluOpType.is_equal)
                for t in range(T):
                    ti = t0i + t
                    (tk, w1, cross) = p.tiles[ti]
                    halves = [(0, w1)] + ([(1, w1 + 1)] if cross else [])
                    for (half, w) in halves:
                        key = (tk, w)
                        if key not in open_ps:
                            open_ps[key] = aggps.tile([128, W], F32, tag="aggw",
                                                      name=f"aw{tk}_{w}_{l}")
                        first, last = p.mm_flags[(ti, half)]
                        rhs = (S[:, t, :] if half == 0
                               else S2[:, p.cross_col[ti] - cx0, :])
                        nc.tensor.matmul(
                            out=open_ps[key][:],
                            lhsT=g[:, t, :],
                            rhs=rhs,
                            start=first, stop=last)
                    for key2 in p.flush_after.get(ti, []):
                        (fk, fw) = key2
                        fq, fwl = fw // NWQ, fw % NWQ
                        sl = slice(fwl * W, (fwl + 1) * W)
                        nc.vector.tensor_tensor(
                            out=aggq[fq][:, sl], in0=open_ps[key2][:],
                            in1=aggq[fq][:, sl], op=mybir.AluOpType.add)
                        del open_ps[key2]
            assert not open_ps
            # mean-divide per quarter
            for q in range(NK):
                qsl = slice(q * PNQ, (q + 1) * PNQ)
                nc.vector.tensor_tensor(out=aggq[q][:], in0=aggq[q][:],
                                        in1=inv_s[:, qsl],
                                        op=mybir.AluOpType.mult)
            es_l.close()

            es_l = ExitStack(); es_l.enter_context(scope(f"upd{l}"))
            qblocks = []
            for q in range(NK):
                off = 0
                while off < PNQ:
                    bw = min(BLK, PNQ - off)
                    qblocks.append((q, off, bw))
                    off += bw
            parts = smallp.tile([128, 2, len(qblocks)], F32, tag="parts")
            for b, (q, off, bw) in enumerate(qblocks):
                gsl = slice(q * PNQ + off, q * PNQ + off + bw)
                z_ps = zps.tile([128, bw], F32, tag="z")
                nc.tensor.matmul(out=z_ps[:], lhsT=wl_s[:, l * D:(l + 1) * D],
                                 rhs=aggq[q][:, off:off + bw], start=True, stop=False)
                nc.tensor.matmul(out=z_ps[:], lhsT=wr_s[:, l * D:(l + 1) * D],
                                 rhs=xt_cur[:, gsl], start=False, stop=True)
                nc.scalar.activation(out=xt_nxt[:, gsl], in_=z_ps[:],
                                     func=mybir.ActivationFunctionType.Copy,
                                     accum_out=parts[:, 0, b:b + 1])
                nc.scalar.activation(out=sq_scr[:, 0:bw], in_=z_ps[:],
                                     func=mybir.ActivationFunctionType.Square,
                                     accum_out=parts[:, 1, b:b + 1])
            es_l.close()

            # BN stats chain entirely on ScalarE so it never queues behind the
            # DVE's one-hot builds (which gate the next layer otherwise)
            es_l = ExitStack(); es_l.enter_context(scope(f"bnred{l}"))
            A = mybir.ActivationFunctionType
            st_loc = smallp.tile([128, 2], F32, tag="stloc")
            red_scr = smallp.tile([128, len(qblocks)], F32, tag="redscr")
            nc.scalar.activation(out=red_scr[:], in_=parts[:, 0, :], func=A.Copy,
                                 accum_out=st_loc[:, 0:1])
            nc.scalar.activation(out=red_scr[:], in_=parts[:, 1, :], func=A.Copy,
                                 accum_out=st_loc[:, 1:2])
            nc.sync.dma_start(out=bnin[l][:], in_=st_loc[:])
            nc.gpsimd.collective_compute(
                "AllReduce", mybir.AluOpType.add, replica_groups=rg,
                ins=[bnin[l][:]], outs=[bnout[l][:]])
            st = smallp.tile([128, 2], F32, tag="st")
            nc.sync.dma_start(out=st[:], in_=bnout[l][:])

            stat = smallp.tile([128, 6], F32, tag="stat")
            inv_n = 1.0 / float(p.N)
            nc.vector.tensor_scalar(out=stat[:, 0:1], in0=st[:, 0:1], scalar1=inv_n,
                                    scalar2=None, op0=mybir.AluOpType.mult)
            nc.vector.tensor_scalar(out=stat[:, 1:2], in0=st[:, 1:2], scalar1=inv_n,
                                    scalar2=None, op0=mybir.AluOpType.mult)
            nc.vector.tensor_tensor(out=stat[:, 2:3], in0=stat[:, 0:1], in1=stat[:, 0:1],
                                    op=mybir.AluOpType.mult)
            nc.vector.tensor_tensor(out=stat[:, 2:3], in0=stat[:, 1:2], in1=stat[:, 2:3],
                                    op=mybir.AluOpType.subtract)
            nc.scalar.activation(out=stat[:, 3:4], in_=stat[:, 2:3],
                                 func=A.Sqrt, bias=eps_s[:, 0:1])
            nc.vector.reciprocal(out=stat[:, 4:5], in_=stat[:, 3:4])
            nc.vector.tensor_tensor(out=stat[:, 4:5], in0=stat[:, 4:5],
                                    in1=gb_s[:, l, 0:1], op=mybir.AluOpType.mult)
            nc.vector.tensor_tensor(out=stat[:, 5:6], in0=stat[:, 0:1], in1=stat[:, 4:5],
                                    op=mybir.AluOpType.mult)
            nc.vector.tensor_tensor(out=stat[:, 5:6], in0=gb_s[:, l, 1:2], in1=stat[:, 5:6],
                                    op=mybir.AluOpType.subtract)
            es_l.close()

            # ---- BN apply + relu per quarter; transpose + AllGather chunk ----
            es_l = ExitStack(); es_l.enter_context(scope(f"bnapp{l}"))
            for q in range(NK):
                qsl = slice(q * PNQ, (q + 1) * PNQ)
                nc.scalar.activation(out=xt_nxt[:, qsl], in_=xt_nxt[:, qsl],
                                     func=mybir.ActivationFunctionType.Relu,
                                     scale=stat[:, 4:5], bias=stat[:, 5:6])
                if q == NK - 1:
                    mt0 = PN - p.MT
                    nc.vector.tensor_tensor(out=xt_nxt[:, mt0:PN],
                                            in0=xt_nxt[:, mt0:PN],
                                            in1=mtail_s[:], op=mybir.AluOpType.mult)
                if l < L - 1:
                    st_t = stage.tile([128, NQB, 128], BF16, tag="stg")
                    for j in range(NQB):
                        col = q * PNQ + j * 128
                        t_ps = tps.tile([128, 128], BF16, tag="tps")
                        nc.tensor.transpose(out=t_ps[:],
                                            in_=xt_nxt[:, col:col + 128],
                                            identity=ident[:])
                        nc.scalar.activation(
                            out=st_t[:, j, :], in_=t_ps[:],
                            func=mybir.ActivationFunctionType.Copy)
                    shard_v = shardq[l][q].ap().rearrange("(t p) d -> p t d", p=128)
                    nc.sync.dma_start(out=shard_v[:], in_=st_t[:])
                    nc.gpsimd.collective_compute(
                        "AllGather", mybir.AluOpType.bypass, replica_groups=rg,
                        ins=[shardq[l][q][:]], outs=[tabs[l + 1][q][:]])
            es_l.close()

        # ---- graph mean pool ----
        es_l = ExitStack(); es_l.enter_context(scope("pool"))
        xt_fin = xt[L % 2]
        pool_ps = zps.tile([128, GPC], F32, tag="z")
        GB = 4   # node blocks per batched one-hot build
        for kb in range(NB):
            t_ps = tps.tile([128, 128], BF16, tag="tps", name=f"tp_pool{kb}")
            nc.tensor.transpose(out=t_ps[:], in_=xt_fin[:, kb * 128:(kb + 1) * 128],
                                identity=ident[:])
            xs = gsel.tile([128, D], BF16, tag="xs")
            nc.vector.tensor_scalar(out=xs[:], in0=t_ps[:],
                                    scalar1=wpool_s[:, kb:kb + 1], scalar2=None,
                                    op0=mybir.AluOpType.mult)
            if kb % GB == 0:
                nb_grp = min(GB, NB - kb)
                Gp = gsel.tile([128, GB, GPC], BF16, tag="Gp")
                nc.vector.tensor_tensor(
                    out=Gp[:, 0:nb_grp, :],
                    in0=bloc_s[:, kb:kb + nb_grp].unsqueeze(-1)
                        .to_broadcast([128, nb_grp, GPC]),
                    in1=iotaG[:].unsqueeze(1).to_broadcast([128, nb_grp, GPC]),
                    op=mybir.AluOpType.is_equal)
            nc.tensor.matmul(out=pool_ps[:], lhsT=xs[:], rhs=Gp[:, kb % GB, :],
                             start=(kb == 0), stop=(kb == NB - 1))
        pool_sb = headp.tile([128, GPC], BF16, tag="poolsb")
        nc.scalar.activation(out=pool_sb[:], in_=pool_ps[:],
                             func=mybir.ActivationFunctionType.Copy)

        h_ps = zps.tile([HD, GPC], F32, tag="z", name="h_ps")
        nc.tensor.matmul(out=h_ps[:], lhsT=w1_s[:], rhs=pool_sb[:], start=True, stop=True)
        h_sb = headp.tile([HD, GPC], BF16, tag="hsb")
        nc.scalar.activation(out=h_sb[:], in_=h_ps[:],
                             func=mybir.ActivationFunctionType.Relu, bias=b1_s[:, 0:1])
        o_ps = zps.tile([1, GPC], F32, tag="z", name="o_ps")
        nc.tensor.matmul(out=o_ps[:], lhsT=w2_s[:], rhs=h_sb[:], start=True, stop=True)
        o_sb = headp.tile([1, GPC], F32, tag="osb")
        nc.vector.tensor_tensor(out=o_sb[:], in0=o_ps[:],
                                in1=b2_s[:].to_broadcast([1, GPC]), op=mybir.AluOpType.add)
        nc.sync.dma_start(out=out_p.ap()[None, :], in_=o_sb[:])
        es_l.close()

    nc.compile()
    return nc


def kernel(**inputs):
    global LAST_RESULT
    x = np.asarray(inputs["x"], np.float32)
    esrc = np.asarray(inputs["edge_src"], np.int64)
    edst = np.asarray(inputs["edge_dst"], np.int64)
    bids = np.asarray(inputs["batch_ids"], np.int64)
    Wl = np.asarray(inputs["Wl"], np.float32)
    Wr = np.asarray(inputs["Wr"], np.float32)
    gamma = np.asarray(inputs["gamma"], np.float32)
    beta = np.asarray(inputs["beta"], np.float32)
    hW1 = np.asarray(inputs["head_W1"], np.float32)
    hb1 = np.asarray(inputs["head_b1"], np.float32)
    hW2 = np.asarray(inputs["head_W2"], np.float32)
    hb2 = np.asarray(inputs["head_b2"], np.float32)

    p = _preprocess(x, esrc, edst, bids)
    nc = _build(p)

    gb = np.stack([gamma.T, beta.T], axis=-1).astype(np.float32)
    shared = {
        "wl": Wl.astype(ml_dtypes.bfloat16),
        "wr": Wr.astype(ml_dtypes.bfloat16),
        "gb": gb,
        "w1": hW1.astype(ml_dtypes.bfloat16),
        "b1": hb1.reshape(HD, 1).astype(np.float32),
        "w2": hW2.astype(ml_dtypes.bfloat16),
        "b2": hb2.reshape(1, 1).astype(np.float32),
    }
    for k in range(NK):
        shared[f"t0_{k}"] = p.t0[k]
    in_maps = []
    for c in range(C):
        m = dict(shared)
        m["xt0"] = p.xt0[c]
        m["eidx"] = p.eidx[c]
        m["ed"] = p.ed[c]
        m["ed2"] = p.ed2[c]
        m["invb"] = p.invb[c]
        m["wpool"] = p.wpool[c]
        m["bloc"] = p.bloc[c]
        m["mtail"] = p.mask_tail[c]
        in_maps.append(m)

    trace = bool(int(os.environ.get("GNN_TRACE", "0")))
    res = run_bass_kernel_spmd(nc, in_maps, core_ids=list(range(C)), trace=trace)
    LAST_RESULT = res
    out = np.concatenate([np.asarray(res.results[c]["out"], np.float32) for c in range(C)])
    return out
